# revision 32
# baseline (speedup 1.0000x reference)
"""Trainium2 Bass kernel for nn_AttentionAugmentation (v2).

Attention with 2D relative-position logits. B=8, H=W=32, dk=dv=256, Nh=8.
Sharding: data-parallel over batch (one batch per NeuronCore, 8 cores).

Per-core algorithm (one batch, 8 heads of 1024x1024 attention, dkh=32):
  - inputs loaded as three column-split casting DMAs (q, k, v) so the q
    transposes / rel-logit matmuls start ~3us in, before k/v land.
  - PE warm-up with real matmuls (transposes don't open the HAM clock gate).
  - q/k PE-transposed in 4-head groups -> DVE copy (q scaled) -> partition-
    scatter DMAs into rows 0-31 of the augmented operands qaug/kaug.
  - relative logits folded into the main QK matmul via a 96-row augmented
    contraction: rows 32-63 (WRELT | one-hot of key y2), rows 64-95
    (HRELT | one-hot of key x2). WRELT/HRELT built with one small matmul
    per (y or x, 4-batch) over all heads; psum batched [32,1024] and copied
    with wide 4D-AP copies split across ACT/DVE (the old per-slice scalar
    copies serialized the whole front half of the kernel).
  - S^T = kaug^T @ qaug per 128-key chunk; exp split between ScalarE
    (true exp) and VectorE (Schraudolph: (int16)(A*S + B) bitcast as bf16
    approximates e^S to ~3%; C tuned for zero mean bias so ACT- and
    DVE-exp'd key chunks agree); AV with lhsT=[V | 1] giving attn^T rows
    plus softmax denominators.
  - attn^T PE-transposed back per 128-query chunk, scaled by reciprocal
    denominators into out_sb; per-pair output DMAs overlap the tail.
Matmul operands bf16 (fp32 PSUM accumulation).
"""
import sys

sys.path.insert(0, "/opt/trn_rl_repo")

from contextlib import ExitStack

import numpy as np

import concourse.bass as bass
from concourse import bacc
import concourse.mybir as mybir
from concourse import masks
from concourse.tile import TileContext

HW = 1024
CH = 768
NH = 8
F32 = mybir.dt.float32
BF16 = mybir.dt.bfloat16
I16 = mybir.dt.int16
EXP = mybir.ActivationFunctionType.Exp
MULT = mybir.AluOpType.mult
ADD = mybir.AluOpType.add
QSCALE = float((256 / 8) ** -0.5)
# Schraudolph exp for bf16: (int16)(A*x + B) bits ~= bf16(e^x).
SCH_A = 184.6649652337873      # 2^7 / ln(2)
SCH_B = 16256.0 - 7.0          # 127 * 2^7 + C (C=-7: zero mean ratio bias)
DVE_CHUNKS = (3, 7)            # key chunks exp'd on VectorE (rest ScalarE)


def build_nc():
    nc = bacc.Bacc()
    # input split in two halves: a single [1024, 768] parameter makes the
    # axon-pjrt reshard program's dynamic-slice exceed a 16-bit semaphore
    # field in neuronx-cc (25MB concat across 8 cores), crashing walrus.
    xa_d = nc.declare_dram_parameter("xa", [HW // 2, CH], F32, isOutput=False)
    xb_d = nc.declare_dram_parameter("xb", [HW // 2, CH], F32, isOutput=False)
    krw_d = nc.declare_dram_parameter("krw", [63, 32], F32, isOutput=False)
    krh_d = nc.declare_dram_parameter("krh", [63, 32], F32, isOutput=False)
    out_d = nc.declare_dram_parameter("out", [HW, 256], F32, isOutput=True)

    with ExitStack() as octx:
        tc = octx.enter_context(TileContext(nc))
        sb = octx.enter_context(tc.tile_pool(name="persist", bufs=1))

        x_sb = sb.tile([128, 6144], BF16)       # (c, e): chunk c, channel e
        qaug = sb.tile([96, NH * HW], BF16)     # per head h: cols 1024h + (32x + y)
        kaug = sb.tile([96, NH * HW], BF16)
        v1 = sb.tile([128, NH * 8 * 33], BF16)  # per (h,c): 33 cols = V chunk | ones
        tscr = sb.tile([128, 4 * HW], BF16)     # transpose scratch (kind, group)
        out_sb = sb.tile([128, 8 * 256], F32)   # col 256c + 32h + d
        identb = sb.tile([128, 128], BF16)
        identf = sb.tile([128, 128], F32)
        krw_sb = sb.tile([63, 32], BF16)
        krh_sb = sb.tile([63, 32], BF16)
        krwT = sb.tile([32, 63], BF16)
        krhT = sb.tile([32, 63], BF16)

        # ---- identity first (gates PE warm-up), then input DMAs ----
        masks.make_identity(nc, identb[:])

        # small rel tables first (krT transposes consume them immediately),
        # then full-bandwidth contiguous chunk loads. Split across two DMA
        # queues: even chunks SWDGE-cast on gpsimd, odd chunks fp32 on sync
        # (HWDGE can't cast) + DVE bf16 casts.
        nc.gpsimd.dma_start(out=krw_sb[:], in_=krw_d[:])
        nc.gpsimd.dma_start(out=krh_sb[:], in_=krh_d[:])
        x32_sb = sb.tile([128, 4 * 768], F32)
        for c in range(8):
            src_d = xa_d if c < 4 else xb_d
            cc = c % 4
            if c % 2 == 0:
                nc.gpsimd.dma_start(
                    out=x_sb[:, 768 * c:768 * c + 768],
                    in_=src_d[128 * cc:128 * cc + 128, :],
                )
            else:
                nc.sync.dma_start(
                    out=x32_sb[:, 768 * (c // 2):768 * (c // 2) + 768],
                    in_=src_d[128 * cc:128 * cc + 128, :],
                )
        for c in (1, 3, 5, 7):
            nc.vector.tensor_copy(
                out=x_sb[:, 768 * c:768 * c + 768],
                in_=x32_sb[:, 768 * (c // 2):768 * (c // 2) + 768],
            )

        # ---- remaining constants (gpsimd queue, after DMA issues) ----
        masks.make_identity(nc, identf[:])
        nc.gpsimd.memset(kaug[32:64, 0:HW], 0.0)
        nc.gpsimd.memset(kaug[64:96, 0:HW], 0.0)
        # one-hot blocks, head-0 only: rows 32-63: [y2(k)==j], rows 64-95:
        # [x2(k)==j]; key col = 32*x2 + y2. DMA-replicated to heads 1-7.
        nc.gpsimd.affine_select(
            out=kaug[32:64, 0:HW].rearrange("p (x y) -> p x y", x=32, y=32),
            in_=kaug[32:64, 0:HW].rearrange("p (x y) -> p x y", x=32, y=32),
            compare_op=mybir.AluOpType.not_equal,
            fill=1.0,
            base=0,
            pattern=[[0, 32], [-1, 32]],
            channel_multiplier=1,
        )
        nc.gpsimd.affine_select(
            out=kaug[64:96, 0:HW].rearrange("p (x y) -> p x y", x=32, y=32),
            in_=kaug[64:96, 0:HW].rearrange("p (x y) -> p x y", x=32, y=32),
            compare_op=mybir.AluOpType.not_equal,
            fill=1.0,
            base=0,
            pattern=[[-1, 32], [0, 32]],
            channel_multiplier=1,
        )
        # one-hot replication on the gpsimd DMA queue (keep sync/scalar free
        # for the latency-critical q/k partition-scatters)
        for base in (32, 64):
            n = HW
            while n < NH * HW:
                rep = min(n, NH * HW - n)
                nc.gpsimd.dma_start(
                    out=kaug[base:base + 32, n:n + rep],
                    in_=kaug[base:base + 32, 0:rep],
                )
                n += rep
        nc.gpsimd.memset(v1[:], 1.0)

        qa4 = qaug[0:32, :].rearrange("p (h x y) -> p h x y", h=8, x=32, y=32)
        wdst = qaug[32:64, :].rearrange("p (h x y) -> p h x y", h=8, x=32, y=32)
        hdst = qaug[64:96, :].rearrange("p (h x y) -> p h x y", h=8, x=32, y=32)

        # ================= Phase A =================
        with ExitStack() as actx:
            psA = actx.enter_context(tc.tile_pool(name="psA", bufs=2, space="PSUM"))

            # filler matmuls woven between transposes / rel batches keep the
            # HAM clock gate open (transposes and sparse small matmuls alone
            # leave the PE at 1.2 GHz). Fillers borrow idle pool slots.
            fillA = psA.tile([32, 1024], F32, tag="rel", bufs=2, name="fillA")

            def filler(ft, n=2):
                for _ in range(n):
                    nc.tensor.matmul(
                        out=ft[0:32, 0:128], lhsT=identb[0:32, 0:32],
                        rhs=identb[0:32, :], start=True, stop=True,
                    )

            # qT/kT psum tiles (kr transposes borrow two slots first)
            pts = {}
            for kind in range(2):
                for g in range(2):
                    pts[(kind, g)] = psA.tile([128, HW], BF16, tag="tps",
                                              bufs=4, name=f"pt{kind}{g}")

            # key_rel transposes: krw [63,32] -> krwT [32,63]. Borrow the tail
            # (chunk-7 region) of two pt slots — those transposes run last,
            # long after the krT copies drain.
            for srct, dst, pt in ((krw_sb, krwT, pts[(0, 0)]),
                                  (krh_sb, krhT, pts[(0, 1)])):
                nc.tensor.transpose(
                    out=pt[0:32, 896:959], in_=srct[:],
                    identity=identb[0:63, 0:63]
                )
                nc.vector.tensor_copy(out=dst[:], in_=pt[0:32, 896:959])

            # qT/kT: PE-transpose 4-head groups per input chunk as it lands,
            # fillers between, then DVE copies + partition-scatter DMAs.
            for c in range(8):
                for kind in range(2):
                    for g in range(2):
                        nc.tensor.transpose(
                            out=pts[(kind, g)][:, 128 * c:128 * c + 128],
                            in_=x_sb[:, 768 * c + 256 * kind + 128 * g:
                                     768 * c + 256 * kind + 128 * g + 128],
                            identity=identb[:, 0:128],
                        )
                filler(fillA, 2)
            for kind in range(2):
                dstt = qaug if kind == 0 else kaug
                for g in range(2):
                    pt = pts[(kind, g)]
                    scr = tscr[:, HW * (2 * kind + g):HW * (2 * kind + g) + HW]
                    if kind == 0:
                        nc.vector.tensor_scalar_mul(out=scr, in0=pt[:], scalar1=QSCALE)
                    else:
                        nc.vector.tensor_copy(out=scr, in_=pt[:])
                    # partition-scatter DMAs split across two issue queues
                    # (sync + gpsimd; NOT scalar - issues there would block
                    # the ACT rel-logit copies behind them in queue order)
                    for hh in range(4):
                        h = 4 * g + hh
                        eng = nc.sync if hh % 2 == 0 else nc.gpsimd
                        eng.dma_start(
                            out=dstt[0:32, HW * h:HW * h + HW],
                            in_=tscr[32 * hh:32 * hh + 32,
                                     HW * (2 * kind + g):HW * (2 * kind + g) + HW],
                        )

            # rel logits, batched: per 4 x-positions (H) / y-positions (W)
            # one [32, 1024] psum + one wide 4D-AP copy. Fillers (into a
            # retired pt slot) keep the PE warm through the copy waits.
            fillB = psA.tile([128, HW], BF16, tag="tps", bufs=4, name="fillB")
            fillBf = fillB[:].bitcast(F32)
            for t in range(8):
                pR = psA.tile([32, 1024], F32, tag="rel", bufs=2)
                pv = pR[:].rearrange("p (i h y) -> p i h y", i=4, h=8, y=32)
                for xi in range(4):
                    x = 4 * t + xi
                    nc.tensor.matmul(
                        out=pv[:, xi, :, :],
                        lhsT=krhT[:, 31 - x:63 - x],
                        rhs=qa4[:, :, x, :],
                        start=True, stop=True,
                    )
                # split each batch copy across both engines (heads 0-3 DVE,
                # 4-7 ACT) so the copy chain doesn't serialize the batches
                dst = hdst[:, :, 4 * t:4 * t + 4, :]
                src = pv[:].rearrange("p i h y -> p h i y")
                nc.vector.tensor_copy(out=dst[:, 0:4], in_=src[:, 0:4])
                nc.scalar.copy(out=dst[:, 4:8], in_=src[:, 4:8])
                filler(fillBf, 2)
            for t in range(8):
                pR = psA.tile([32, 1024], F32, tag="rel", bufs=2)
                pw = pR[:].rearrange("p (i h x) -> p i h x", i=4, h=8, x=32)
                for yi in range(4):
                    y = 4 * t + yi
                    nc.tensor.matmul(
                        out=pw[:, yi, :, :],
                        lhsT=krwT[:, 31 - y:63 - y],
                        rhs=qa4[:, :, :, y],
                        start=True, stop=True,
                    )
                dst = wdst[:, :, :, 4 * t:4 * t + 4]
                src = pw[:].rearrange("p i h x -> p h x i")
                nc.vector.tensor_copy(out=dst[:, 0:6], in_=src[:, 0:6])
                nc.scalar.copy(out=dst[:, 6:8], in_=src[:, 6:8])
                filler(fillBf, 2)

            # V chunks into v1 via SBUF->SBUF DMAs (64B runs)
            v1v = v1[:].rearrange("p (h c e) -> p c h e", h=8, c=8, e=33)
            for c in range(8):
                nc.sync.dma_start(
                    out=v1v[:, c, :, 0:32],
                    in_=x_sb[:, 768 * c + 512:768 * c + 768]
                        .rearrange("p (h e) -> p h e", h=8),
                )

        # ================= Phase B: attention =================
        with ExitStack() as bctx:
            psB = bctx.enter_context(tc.tile_pool(name="psB", bufs=1, space="PSUM"))
            sbW = bctx.enter_context(tc.tile_pool(name="sbW", bufs=3))
            sbA = bctx.enter_context(tc.tile_pool(name="sbA", bufs=2))
            sbR = bctx.enter_context(tc.tile_pool(name="sbR", bufs=3))

            def pair_chunks(hp, out_cb=None):
                """QK -> exp -> AV for the two heads of pair hp; returns the
                att psum -> SBUF staging tiles (copies emitted here so the
                att psum slots free up for the next pair ASAP). out_cb(i)
                (i=0..15) interleaves the previous pair's output tail one
                transpose per chunk so it never blocks the PE FIFO."""
                att = [psB.tile([97, 512], F32, tag=f"att{e}", bufs=1,
                                name=f"att{e}")
                       for e in range(2)]
                for hh in range(2):
                    h = 2 * hp + hh
                    pb = 64 * hh
                    for c in range(8):
                        if out_cb is not None:
                            out_cb(8 * hh + c)
                        s_ps = psB.tile([128, HW], F32, tag="s", bufs=2)
                        for e in range(2):
                            nc.tensor.matmul(
                                out=s_ps[:, 512 * e:512 * e + 512],
                                lhsT=kaug[:, HW * h + 128 * c:HW * h + 128 * c + 128],
                                rhs=qaug[:, HW * h + 512 * e:HW * h + 512 * e + 512],
                                start=True, stop=True,
                            )
                        wexp = sbW.tile([128, HW], BF16, tag="wexp")
                        if c in DVE_CHUNKS:
                            nc.vector.tensor_scalar(
                                out=wexp[:].bitcast(I16),
                                in0=s_ps[:],
                                scalar1=SCH_A, scalar2=SCH_B,
                                op0=MULT, op1=ADD,
                            )
                        else:
                            nc.scalar.activation(out=wexp[:], in_=s_ps[:], func=EXP)
                        for e in range(2):
                            nc.tensor.matmul(
                                out=att[e][pb:pb + 33, :],
                                lhsT=v1[:, 264 * h + 33 * c:264 * h + 33 * c + 33],
                                rhs=wexp[:, 512 * e:512 * e + 512],
                                start=(c == 0), stop=(c == 7),
                            )
                att_sb = []
                for e in range(2):
                    asb = sbA.tile([97, 512], F32, tag="attsb", bufs=4,
                                   name=f"attsb{e}")
                    nc.vector.tensor_copy(out=asb[:], in_=att[e][:])
                    att_sb.append(asb)
                return att_sb

            def pair_output_step(hp, att_sb, i):
                """One transpose + scale step (i=0..7) of pair hp's tail."""
                e, ci = i // 4, i % 4
                c = 4 * e + ci
                ot = psB.tile([128, 97], F32, tag="ot", bufs=2)
                nc.tensor.transpose(
                    out=ot[:],
                    in_=att_sb[e][:, 128 * ci:128 * ci + 128],
                    identity=identf[0:97, 0:97],
                )
                rc = sbR.tile([128, 2], F32, tag="rc")
                nc.vector.reciprocal(out=rc[:], in_=ot[:, 32:97:64])
                for hh in range(2):
                    nc.vector.tensor_scalar_mul(
                        out=out_sb[:, 256 * c + 64 * hp + 32 * hh:
                                   256 * c + 64 * hp + 32 * hh + 32],
                        in0=ot[:, 64 * hh:64 * hh + 32],
                        scalar1=rc[:, hh:hh + 1],
                    )
                if i == 7:
                    # per-pair output DMA (cols 64hp..+63 of each 256-block)
                    nc.sync.dma_start(
                        out=out_d[:].rearrange("(c p) d -> p c d", p=128)
                            [:, :, 64 * hp:64 * hp + 64],
                        in_=out_sb[:].rearrange("p (c d) -> p c d", c=8)
                            [:, :, 64 * hp:64 * hp + 64],
                    )

            pending = None
            for hp in range(NH // 2):
                prev = pending

                def out_cb(i, p=prev):
                    if p is not None and i % 2 == 0:
                        pair_output_step(p[0], p[1], i // 2)

                att_sb = pair_chunks(hp, out_cb=out_cb if prev else None)
                pending = (hp, att_sb)
            for i in range(8):
                pair_output_step(pending[0], pending[1], i)
    if not nc.is_finalized():
        nc.finalize()
    return nc


_NC = None


def _ensure_axon_hooks_module():
    """bass_utils imports antenv.axon_hooks unconditionally when trace=True;
    this image's antenv lacks it. Provide a stub so tracing degrades to
    no-trace instead of crashing (a real hook can be set by a profiler)."""
    import types

    if "antenv.axon_hooks" in sys.modules:
        return
    try:
        import antenv.axon_hooks  # noqa: F401
        return
    except ImportError:
        pass
    try:
        import antenv
    except ImportError:
        return
    m = types.ModuleType("antenv.axon_hooks")
    m._hook = None
    m.get_axon_ntff_profile_hook = lambda: m._hook
    m.set_axon_ntff_profile_hook = lambda h: setattr(m, "_hook", h)
    sys.modules["antenv.axon_hooks"] = m
    antenv.axon_hooks = m


def kernel(**inputs):
    global _NC
    x = np.ascontiguousarray(np.asarray(inputs["inputs"], dtype=np.float32))
    krw = np.ascontiguousarray(np.asarray(inputs["key_rel_w"], dtype=np.float32))
    krh = np.ascontiguousarray(np.asarray(inputs["key_rel_h"], dtype=np.float32))
    assert x.shape == (8, 32, 32, 768), x.shape
    assert int(inputs["dk"]) == 256 and int(inputs["dv"]) == 256
    assert int(inputs["Nh"]) == 8

    if _NC is None:
        _NC = build_nc()
    _ensure_axon_hooks_module()
    from concourse.bass_utils import run_bass_kernel_spmd

    in_maps = [
        {
            "xa": x[b].reshape(HW, CH)[:HW // 2],
            "xb": x[b].reshape(HW, CH)[HW // 2:],
            "krw": krw,
            "krh": krh,
        }
        for b in range(8)
    ]
    res = run_bass_kernel_spmd(_NC, in_maps, list(range(8)))
    kernel.last_result = res
    out = np.stack([res.results[b]["out"].reshape(32, 32, 256) for b in range(8)], 0)
    return out


if __name__ == "__main__":
    nc = build_nc()
    print("built ok")


# revision 34
# speedup vs baseline: 1.0470x; 1.0470x over previous
"""Trainium2 Bass kernel for nn_AttentionAugmentation (v2).

Attention with 2D relative-position logits. B=8, H=W=32, dk=dv=256, Nh=8.
Sharding: data-parallel over batch (one batch per NeuronCore, 8 cores).

Per-core algorithm (one batch, 8 heads of 1024x1024 attention, dkh=32):
  - inputs loaded as three column-split casting DMAs (q, k, v) so the q
    transposes / rel-logit matmuls start ~3us in, before k/v land.
  - PE warm-up with real matmuls (transposes don't open the HAM clock gate).
  - q/k PE-transposed in 4-head groups -> DVE copy (q scaled) -> partition-
    scatter DMAs into rows 0-31 of the augmented operands qaug/kaug.
  - relative logits folded into the main QK matmul via a 96-row augmented
    contraction: rows 32-63 (WRELT | one-hot of key y2), rows 64-95
    (HRELT | one-hot of key x2). WRELT/HRELT built with one small matmul
    per (y or x, 4-batch) over all heads; psum batched [32,1024] and copied
    with wide 4D-AP copies split across ACT/DVE (the old per-slice scalar
    copies serialized the whole front half of the kernel).
  - S^T = kaug^T @ qaug per 128-key chunk; exp split between ScalarE
    (true exp) and VectorE (Schraudolph: (int16)(A*S + B) bitcast as bf16
    approximates e^S to ~3%; C tuned for zero mean bias so ACT- and
    DVE-exp'd key chunks agree); AV with lhsT=[V | 1] giving attn^T rows
    plus softmax denominators.
  - attn^T PE-transposed back per 128-query chunk, scaled by reciprocal
    denominators into out_sb; per-pair output DMAs overlap the tail.
Matmul operands bf16 (fp32 PSUM accumulation).
"""
import sys

sys.path.insert(0, "/opt/trn_rl_repo")

from contextlib import ExitStack

import numpy as np

import concourse.bass as bass
from concourse import bacc
import concourse.mybir as mybir
from concourse import masks
from concourse.tile import TileContext

HW = 1024
CH = 768
NH = 8
F32 = mybir.dt.float32
BF16 = mybir.dt.bfloat16
I16 = mybir.dt.int16
EXP = mybir.ActivationFunctionType.Exp
MULT = mybir.AluOpType.mult
ADD = mybir.AluOpType.add
QSCALE = float((256 / 8) ** -0.5)
# Schraudolph exp for bf16: (int16)(A*x + B) bits ~= bf16(e^x).
SCH_A = 184.6649652337873      # 2^7 / ln(2)
SCH_B = 16256.0 - 7.0          # 127 * 2^7 + C (C=-7: zero mean ratio bias)
DVE_CHUNKS = (3, 7)            # key chunks exp'd on VectorE (rest ScalarE)


def build_nc():
    nc = bacc.Bacc()
    # input split in two halves: a single [1024, 768] parameter makes the
    # axon-pjrt reshard program's dynamic-slice exceed a 16-bit semaphore
    # field in neuronx-cc (25MB concat across 8 cores), crashing walrus.
    xa_d = nc.declare_dram_parameter("xa", [HW // 2, CH], F32, isOutput=False)
    xb_d = nc.declare_dram_parameter("xb", [HW // 2, CH], F32, isOutput=False)
    krw_d = nc.declare_dram_parameter("krw", [63, 32], F32, isOutput=False)
    krh_d = nc.declare_dram_parameter("krh", [63, 32], F32, isOutput=False)
    out_d = nc.declare_dram_parameter("out", [HW, 256], F32, isOutput=True)

    with ExitStack() as octx:
        tc = octx.enter_context(TileContext(nc))
        sb = octx.enter_context(tc.tile_pool(name="persist", bufs=1))

        x_sb = sb.tile([128, 6144], BF16)       # (c, e): chunk c, channel e
        qaug = sb.tile([96, NH * HW], BF16)     # per head h: cols 1024h + (32x + y)
        kaug = sb.tile([96, NH * HW], BF16)
        v1 = sb.tile([128, NH * 8 * 33], BF16)  # per (h,c): 33 cols = V chunk | ones
        tscr = sb.tile([128, 4 * HW], BF16)     # transpose scratch (kind, group)
        out_sb = sb.tile([128, 8 * 256], F32)   # col 256c + 32h + d
        identb = sb.tile([128, 128], BF16)
        identf = sb.tile([128, 128], F32)
        krw_sb = sb.tile([63, 32], BF16)
        krh_sb = sb.tile([63, 32], BF16)
        krwT = sb.tile([32, 63], BF16)
        krhT = sb.tile([32, 63], BF16)

        # ---- identity first (gates PE warm-up), then input DMAs ----
        masks.make_identity(nc, identb[:])

        # small rel tables first (krT transposes consume them immediately),
        # then full-bandwidth contiguous chunk loads. Split across two DMA
        # queues: even chunks SWDGE-cast on gpsimd, odd chunks fp32 on sync
        # (HWDGE can't cast) + DVE bf16 casts.
        nc.gpsimd.dma_start(out=krw_sb[:], in_=krw_d[:])
        nc.gpsimd.dma_start(out=krh_sb[:], in_=krh_d[:])
        x32_sb = sb.tile([128, 4 * 768], F32)
        for c in range(8):
            src_d = xa_d if c < 4 else xb_d
            cc = c % 4
            if c % 2 == 0:
                nc.gpsimd.dma_start(
                    out=x_sb[:, 768 * c:768 * c + 768],
                    in_=src_d[128 * cc:128 * cc + 128, :],
                )
            else:
                nc.sync.dma_start(
                    out=x32_sb[:, 768 * (c // 2):768 * (c // 2) + 768],
                    in_=src_d[128 * cc:128 * cc + 128, :],
                )
        for c in (1, 3, 5, 7):
            nc.vector.tensor_copy(
                out=x_sb[:, 768 * c:768 * c + 768],
                in_=x32_sb[:, 768 * (c // 2):768 * (c // 2) + 768],
            )

        # ---- remaining constants (gpsimd queue, after DMA issues) ----
        masks.make_identity(nc, identf[:])
        nc.gpsimd.memset(kaug[32:64, 0:HW], 0.0)
        nc.gpsimd.memset(kaug[64:96, 0:HW], 0.0)
        # one-hot blocks, head-0 only: rows 32-63: [y2(k)==j], rows 64-95:
        # [x2(k)==j]; key col = 32*x2 + y2. DMA-replicated to heads 1-7.
        nc.gpsimd.affine_select(
            out=kaug[32:64, 0:HW].rearrange("p (x y) -> p x y", x=32, y=32),
            in_=kaug[32:64, 0:HW].rearrange("p (x y) -> p x y", x=32, y=32),
            compare_op=mybir.AluOpType.not_equal,
            fill=1.0,
            base=0,
            pattern=[[0, 32], [-1, 32]],
            channel_multiplier=1,
        )
        nc.gpsimd.affine_select(
            out=kaug[64:96, 0:HW].rearrange("p (x y) -> p x y", x=32, y=32),
            in_=kaug[64:96, 0:HW].rearrange("p (x y) -> p x y", x=32, y=32),
            compare_op=mybir.AluOpType.not_equal,
            fill=1.0,
            base=0,
            pattern=[[-1, 32], [0, 32]],
            channel_multiplier=1,
        )
        nc.gpsimd.memset(v1[:], 1.0)

        def emit_onehot_replication():
            # one-hot replication on the gpsimd DMA queue, emitted AFTER the
            # q/k partition-scatters so its 14 issues don't delay them
            # (needed only by the first QK matmul, much later).
            for base in (32, 64):
                n = HW
                while n < NH * HW:
                    rep = min(n, NH * HW - n)
                    nc.gpsimd.dma_start(
                        out=kaug[base:base + 32, n:n + rep],
                        in_=kaug[base:base + 32, 0:rep],
                    )
                    n += rep

        qa4 = qaug[0:32, :].rearrange("p (h x y) -> p h x y", h=8, x=32, y=32)
        wdst = qaug[32:64, :].rearrange("p (h x y) -> p h x y", h=8, x=32, y=32)
        hdst = qaug[64:96, :].rearrange("p (h x y) -> p h x y", h=8, x=32, y=32)

        # ================= Phase A =================
        with ExitStack() as actx:
            psA = actx.enter_context(tc.tile_pool(name="psA", bufs=2, space="PSUM"))

            # filler matmuls woven between transposes / rel batches keep the
            # HAM clock gate open (transposes and sparse small matmuls alone
            # leave the PE at 1.2 GHz). Fillers borrow idle pool slots.
            fillA = psA.tile([32, 1024], F32, tag="rel", bufs=2, name="fillA")

            def filler(ft, n=2):
                for _ in range(n):
                    nc.tensor.matmul(
                        out=ft[0:32, 0:128], lhsT=identb[0:32, 0:32],
                        rhs=identb[0:32, :], start=True, stop=True,
                    )

            # qT/kT psum tiles (kr transposes borrow two slots first)
            pts = {}
            for kind in range(2):
                for g in range(2):
                    pts[(kind, g)] = psA.tile([128, HW], BF16, tag="tps",
                                              bufs=4, name=f"pt{kind}{g}")

            # key_rel transposes: krw [63,32] -> krwT [32,63]. Borrow the tail
            # (chunk-7 region) of two pt slots — those transposes run last,
            # long after the krT copies drain.
            for srct, dst, pt in ((krw_sb, krwT, pts[(0, 0)]),
                                  (krh_sb, krhT, pts[(0, 1)])):
                nc.tensor.transpose(
                    out=pt[0:32, 896:959], in_=srct[:],
                    identity=identb[0:63, 0:63]
                )
                nc.vector.tensor_copy(out=dst[:], in_=pt[0:32, 896:959])

            # qT/kT: PE-transpose 4-head groups per input chunk as it lands,
            # fillers between, then DVE copies + partition-scatter DMAs.
            for c in range(8):
                for kind in range(2):
                    for g in range(2):
                        nc.tensor.transpose(
                            out=pts[(kind, g)][:, 128 * c:128 * c + 128],
                            in_=x_sb[:, 768 * c + 256 * kind + 128 * g:
                                     768 * c + 256 * kind + 128 * g + 128],
                            identity=identb[:, 0:128],
                        )
                filler(fillA, 2)
            for kind in range(2):
                dstt = qaug if kind == 0 else kaug
                for g in range(2):
                    pt = pts[(kind, g)]
                    scr = tscr[:, HW * (2 * kind + g):HW * (2 * kind + g) + HW]
                    if kind == 0:
                        nc.vector.tensor_scalar_mul(out=scr, in0=pt[:], scalar1=QSCALE)
                    else:
                        nc.vector.tensor_copy(out=scr, in_=pt[:])
                    # partition-scatter DMAs split across two issue queues
                    # (sync + gpsimd; NOT scalar - issues there would block
                    # the ACT rel-logit copies behind them in queue order)
                    for hh in range(4):
                        h = 4 * g + hh
                        eng = nc.sync if hh % 2 == 0 else nc.gpsimd
                        eng.dma_start(
                            out=dstt[0:32, HW * h:HW * h + HW],
                            in_=tscr[32 * hh:32 * hh + 32,
                                     HW * (2 * kind + g):HW * (2 * kind + g) + HW],
                        )
            emit_onehot_replication()

            # rel logits, batched: per 4 x-positions (H) / y-positions (W)
            # one [32, 1024] psum + one wide 4D-AP copy. Fillers (into a
            # retired pt slot) keep the PE warm through the copy waits.
            fillB = psA.tile([128, HW], BF16, tag="tps", bufs=4, name="fillB")
            fillBf = fillB[:].bitcast(F32)
            for t in range(8):
                pR = psA.tile([32, 1024], F32, tag="rel", bufs=2)
                pv = pR[:].rearrange("p (i h y) -> p i h y", i=4, h=8, y=32)
                for xi in range(4):
                    x = 4 * t + xi
                    nc.tensor.matmul(
                        out=pv[:, xi, :, :],
                        lhsT=krhT[:, 31 - x:63 - x],
                        rhs=qa4[:, :, x, :],
                        start=True, stop=True,
                    )
                # split each batch copy across both engines (heads 0-3 DVE,
                # 4-7 ACT) so the copy chain doesn't serialize the batches
                dst = hdst[:, :, 4 * t:4 * t + 4, :]
                src = pv[:].rearrange("p i h y -> p h i y")
                nc.vector.tensor_copy(out=dst[:, 0:4], in_=src[:, 0:4])
                nc.scalar.copy(out=dst[:, 4:8], in_=src[:, 4:8])
                filler(fillBf, 2)
            for t in range(8):
                pR = psA.tile([32, 1024], F32, tag="rel", bufs=2)
                pw = pR[:].rearrange("p (i h x) -> p i h x", i=4, h=8, x=32)
                for yi in range(4):
                    y = 4 * t + yi
                    nc.tensor.matmul(
                        out=pw[:, yi, :, :],
                        lhsT=krwT[:, 31 - y:63 - y],
                        rhs=qa4[:, :, :, y],
                        start=True, stop=True,
                    )
                dst = wdst[:, :, :, 4 * t:4 * t + 4]
                src = pw[:].rearrange("p i h x -> p h x i")
                nc.vector.tensor_copy(out=dst[:, 0:6], in_=src[:, 0:6])
                nc.scalar.copy(out=dst[:, 6:8], in_=src[:, 6:8])
                filler(fillBf, 2)

            # V chunks into v1 via SBUF->SBUF DMAs (64B runs)
            v1v = v1[:].rearrange("p (h c e) -> p c h e", h=8, c=8, e=33)
            for c in range(8):
                nc.sync.dma_start(
                    out=v1v[:, c, :, 0:32],
                    in_=x_sb[:, 768 * c + 512:768 * c + 768]
                        .rearrange("p (h e) -> p h e", h=8),
                )

        # ================= Phase B: attention =================
        with ExitStack() as bctx:
            psB = bctx.enter_context(tc.tile_pool(name="psB", bufs=1, space="PSUM"))
            sbW = bctx.enter_context(tc.tile_pool(name="sbW", bufs=3))
            sbA = bctx.enter_context(tc.tile_pool(name="sbA", bufs=2))
            sbR = bctx.enter_context(tc.tile_pool(name="sbR", bufs=3))

            def pair_chunks(hp, out_cb=None):
                """QK -> exp -> AV for the two heads of pair hp; returns the
                att psum -> SBUF staging tiles (copies emitted here so the
                att psum slots free up for the next pair ASAP). out_cb(i)
                (i=0..15) interleaves the previous pair's output tail one
                transpose per chunk so it never blocks the PE FIFO."""
                att = [psB.tile([97, 512], F32, tag=f"att{e}", bufs=1,
                                name=f"att{e}")
                       for e in range(2)]
                for hh in range(2):
                    h = 2 * hp + hh
                    pb = 64 * hh
                    for c in range(8):
                        if out_cb is not None:
                            out_cb(8 * hh + c)
                        s_ps = psB.tile([128, HW], F32, tag="s", bufs=2)
                        for e in range(2):
                            nc.tensor.matmul(
                                out=s_ps[:, 512 * e:512 * e + 512],
                                lhsT=kaug[:, HW * h + 128 * c:HW * h + 128 * c + 128],
                                rhs=qaug[:, HW * h + 512 * e:HW * h + 512 * e + 512],
                                start=True, stop=True,
                            )
                        wexp = sbW.tile([128, HW], BF16, tag="wexp")
                        if c in DVE_CHUNKS:
                            nc.vector.tensor_scalar(
                                out=wexp[:].bitcast(I16),
                                in0=s_ps[:],
                                scalar1=SCH_A, scalar2=SCH_B,
                                op0=MULT, op1=ADD,
                            )
                        else:
                            nc.scalar.activation(out=wexp[:], in_=s_ps[:], func=EXP)
                        for e in range(2):
                            nc.tensor.matmul(
                                out=att[e][pb:pb + 33, :],
                                lhsT=v1[:, 264 * h + 33 * c:264 * h + 33 * c + 33],
                                rhs=wexp[:, 512 * e:512 * e + 512],
                                start=(c == 0), stop=(c == 7),
                            )
                att_sb = []
                for e in range(2):
                    asb = sbA.tile([97, 512], F32, tag="attsb", bufs=4,
                                   name=f"attsb{e}")
                    nc.vector.tensor_copy(out=asb[:], in_=att[e][:])
                    att_sb.append(asb)
                return att_sb

            def pair_output_step(hp, att_sb, i):
                """One transpose + scale step (i=0..7) of pair hp's tail."""
                e, ci = i // 4, i % 4
                c = 4 * e + ci
                ot = psB.tile([128, 97], F32, tag="ot", bufs=2)
                nc.tensor.transpose(
                    out=ot[:],
                    in_=att_sb[e][:, 128 * ci:128 * ci + 128],
                    identity=identf[0:97, 0:97],
                )
                rc = sbR.tile([128, 2], F32, tag="rc")
                nc.vector.reciprocal(out=rc[:], in_=ot[:, 32:97:64])
                for hh in range(2):
                    nc.vector.tensor_scalar_mul(
                        out=out_sb[:, 256 * c + 64 * hp + 32 * hh:
                                   256 * c + 64 * hp + 32 * hh + 32],
                        in0=ot[:, 64 * hh:64 * hh + 32],
                        scalar1=rc[:, hh:hh + 1],
                    )
                if i == 7:
                    # per-pair output DMA (cols 64hp..+63 of each 256-block)
                    nc.sync.dma_start(
                        out=out_d[:].rearrange("(c p) d -> p c d", p=128)
                            [:, :, 64 * hp:64 * hp + 64],
                        in_=out_sb[:].rearrange("p (c d) -> p c d", c=8)
                            [:, :, 64 * hp:64 * hp + 64],
                    )

            pending = None
            for hp in range(NH // 2):
                prev = pending

                def out_cb(i, p=prev):
                    if p is not None and i % 2 == 0:
                        pair_output_step(p[0], p[1], i // 2)

                att_sb = pair_chunks(hp, out_cb=out_cb if prev else None)
                pending = (hp, att_sb)
            for i in range(8):
                pair_output_step(pending[0], pending[1], i)
    if not nc.is_finalized():
        nc.finalize()
    return nc


_NC = None


def _ensure_axon_hooks_module():
    """bass_utils imports antenv.axon_hooks unconditionally when trace=True;
    this image's antenv lacks it. Provide a stub so tracing degrades to
    no-trace instead of crashing (a real hook can be set by a profiler)."""
    import types

    if "antenv.axon_hooks" in sys.modules:
        return
    try:
        import antenv.axon_hooks  # noqa: F401
        return
    except ImportError:
        pass
    try:
        import antenv
    except ImportError:
        return
    m = types.ModuleType("antenv.axon_hooks")
    m._hook = None
    m.get_axon_ntff_profile_hook = lambda: m._hook
    m.set_axon_ntff_profile_hook = lambda h: setattr(m, "_hook", h)
    sys.modules["antenv.axon_hooks"] = m
    antenv.axon_hooks = m


def kernel(**inputs):
    global _NC
    x = np.ascontiguousarray(np.asarray(inputs["inputs"], dtype=np.float32))
    krw = np.ascontiguousarray(np.asarray(inputs["key_rel_w"], dtype=np.float32))
    krh = np.ascontiguousarray(np.asarray(inputs["key_rel_h"], dtype=np.float32))
    assert x.shape == (8, 32, 32, 768), x.shape
    assert int(inputs["dk"]) == 256 and int(inputs["dv"]) == 256
    assert int(inputs["Nh"]) == 8

    if _NC is None:
        _NC = build_nc()
    _ensure_axon_hooks_module()
    from concourse.bass_utils import run_bass_kernel_spmd

    in_maps = [
        {
            "xa": x[b].reshape(HW, CH)[:HW // 2],
            "xb": x[b].reshape(HW, CH)[HW // 2:],
            "krw": krw,
            "krh": krh,
        }
        for b in range(8)
    ]
    res = run_bass_kernel_spmd(_NC, in_maps, list(range(8)))
    kernel.last_result = res
    out = np.stack([res.results[b]["out"].reshape(32, 32, 256) for b in range(8)], 0)
    return out


if __name__ == "__main__":
    nc = build_nc()
    print("built ok")


# revision 35
# speedup vs baseline: 1.0590x; 1.0114x over previous
"""Trainium2 Bass kernel for nn_AttentionAugmentation (v2).

Attention with 2D relative-position logits. B=8, H=W=32, dk=dv=256, Nh=8.
Sharding: data-parallel over batch (one batch per NeuronCore, 8 cores).

Per-core algorithm (one batch, 8 heads of 1024x1024 attention, dkh=32):
  - inputs loaded as three column-split casting DMAs (q, k, v) so the q
    transposes / rel-logit matmuls start ~3us in, before k/v land.
  - PE warm-up with real matmuls (transposes don't open the HAM clock gate).
  - q/k PE-transposed in 4-head groups -> DVE copy (q scaled) -> partition-
    scatter DMAs into rows 0-31 of the augmented operands qaug/kaug.
  - relative logits folded into the main QK matmul via a 96-row augmented
    contraction: rows 32-63 (WRELT | one-hot of key y2), rows 64-95
    (HRELT | one-hot of key x2). WRELT/HRELT built with one small matmul
    per (y or x, 4-batch) over all heads; psum batched [32,1024] and copied
    with wide 4D-AP copies split across ACT/DVE (the old per-slice scalar
    copies serialized the whole front half of the kernel).
  - S^T = kaug^T @ qaug per 128-key chunk; exp split between ScalarE
    (true exp) and VectorE (Schraudolph: (int16)(A*S + B) bitcast as bf16
    approximates e^S to ~3%; C tuned for zero mean bias so ACT- and
    DVE-exp'd key chunks agree); AV with lhsT=[V | 1] giving attn^T rows
    plus softmax denominators.
  - attn^T PE-transposed back per 128-query chunk, scaled by reciprocal
    denominators into out_sb; per-pair output DMAs overlap the tail.
Matmul operands bf16 (fp32 PSUM accumulation).
"""
import sys

sys.path.insert(0, "/opt/trn_rl_repo")

from contextlib import ExitStack

import numpy as np

import concourse.bass as bass
from concourse import bacc
import concourse.mybir as mybir
from concourse import masks
from concourse.tile import TileContext

HW = 1024
CH = 768
NH = 8
F32 = mybir.dt.float32
BF16 = mybir.dt.bfloat16
I16 = mybir.dt.int16
EXP = mybir.ActivationFunctionType.Exp
MULT = mybir.AluOpType.mult
ADD = mybir.AluOpType.add
QSCALE = float((256 / 8) ** -0.5)
# Schraudolph exp for bf16: (int16)(A*x + B) bits ~= bf16(e^x).
SCH_A = 184.6649652337873      # 2^7 / ln(2)
SCH_B = 16256.0 - 7.0          # 127 * 2^7 + C (C=-7: zero mean ratio bias)
DVE_CHUNKS = (3, 7)            # key chunks exp'd on VectorE (rest ScalarE)


def build_nc():
    nc = bacc.Bacc()
    # input split in two halves: a single [1024, 768] parameter makes the
    # axon-pjrt reshard program's dynamic-slice exceed a 16-bit semaphore
    # field in neuronx-cc (25MB concat across 8 cores), crashing walrus.
    xa_d = nc.declare_dram_parameter("xa", [HW // 2, CH], F32, isOutput=False)
    xb_d = nc.declare_dram_parameter("xb", [HW // 2, CH], F32, isOutput=False)
    krw_d = nc.declare_dram_parameter("krw", [63, 32], F32, isOutput=False)
    krh_d = nc.declare_dram_parameter("krh", [63, 32], F32, isOutput=False)
    out_d = nc.declare_dram_parameter("out", [HW, 256], F32, isOutput=True)

    with ExitStack() as octx:
        tc = octx.enter_context(TileContext(nc))
        sb = octx.enter_context(tc.tile_pool(name="persist", bufs=1))

        x_sb = sb.tile([128, 6144], BF16)       # (c, e): chunk c, channel e
        qaug = sb.tile([96, NH * HW], BF16)     # per head h: cols 1024h + (32x + y)
        kaug = sb.tile([96, NH * HW], BF16)
        v1 = sb.tile([128, NH * 8 * 33], BF16)  # per (h,c): 33 cols = V chunk | ones
        tscr = sb.tile([128, 4 * HW], BF16)     # transpose scratch (kind, group)
        out_sb = sb.tile([128, 8 * 256], F32)   # col 256c + 32h + d
        identb = sb.tile([128, 128], BF16)
        identf = sb.tile([128, 128], F32)
        krw_sb = sb.tile([63, 32], BF16)
        krh_sb = sb.tile([63, 32], BF16)
        krwT = sb.tile([32, 63], BF16)
        krhT = sb.tile([32, 63], BF16)

        # ---- identity first (gates PE warm-up), then input DMAs ----
        masks.make_identity(nc, identb[:])

        # small rel tables first (krT transposes consume them immediately),
        # then full-bandwidth contiguous chunk loads. Split across two DMA
        # queues: even chunks SWDGE-cast on gpsimd, odd chunks fp32 on sync
        # (HWDGE can't cast) + DVE bf16 casts.
        nc.gpsimd.dma_start(out=krw_sb[:], in_=krw_d[:])
        nc.gpsimd.dma_start(out=krh_sb[:], in_=krh_d[:])
        x32_sb = sb.tile([128, 4 * 768], F32)
        for c in range(8):
            src_d = xa_d if c < 4 else xb_d
            cc = c % 4
            if c % 2 == 0:
                nc.gpsimd.dma_start(
                    out=x_sb[:, 768 * c:768 * c + 768],
                    in_=src_d[128 * cc:128 * cc + 128, :],
                )
            else:
                nc.sync.dma_start(
                    out=x32_sb[:, 768 * (c // 2):768 * (c // 2) + 768],
                    in_=src_d[128 * cc:128 * cc + 128, :],
                )
        for c in (1, 3, 5, 7):
            nc.vector.tensor_copy(
                out=x_sb[:, 768 * c:768 * c + 768],
                in_=x32_sb[:, 768 * (c // 2):768 * (c // 2) + 768],
            )

        # ---- remaining constants (gpsimd queue, after DMA issues) ----
        masks.make_identity(nc, identf[:])
        nc.gpsimd.memset(kaug[32:64, 0:HW], 0.0)
        nc.gpsimd.memset(kaug[64:96, 0:HW], 0.0)
        # one-hot blocks, head-0 only: rows 32-63: [y2(k)==j], rows 64-95:
        # [x2(k)==j]; key col = 32*x2 + y2. DMA-replicated to heads 1-7.
        nc.gpsimd.affine_select(
            out=kaug[32:64, 0:HW].rearrange("p (x y) -> p x y", x=32, y=32),
            in_=kaug[32:64, 0:HW].rearrange("p (x y) -> p x y", x=32, y=32),
            compare_op=mybir.AluOpType.not_equal,
            fill=1.0,
            base=0,
            pattern=[[0, 32], [-1, 32]],
            channel_multiplier=1,
        )
        nc.gpsimd.affine_select(
            out=kaug[64:96, 0:HW].rearrange("p (x y) -> p x y", x=32, y=32),
            in_=kaug[64:96, 0:HW].rearrange("p (x y) -> p x y", x=32, y=32),
            compare_op=mybir.AluOpType.not_equal,
            fill=1.0,
            base=0,
            pattern=[[-1, 32], [0, 32]],
            channel_multiplier=1,
        )
        nc.gpsimd.memset(v1[:], 1.0)

        def emit_onehot_replication():
            # one-hot replication on the gpsimd DMA queue, emitted AFTER the
            # q/k partition-scatters so its 14 issues don't delay them
            # (needed only by the first QK matmul, much later).
            for base in (32, 64):
                n = HW
                while n < NH * HW:
                    rep = min(n, NH * HW - n)
                    nc.gpsimd.dma_start(
                        out=kaug[base:base + 32, n:n + rep],
                        in_=kaug[base:base + 32, 0:rep],
                    )
                    n += rep

        qa4 = qaug[0:32, :].rearrange("p (h x y) -> p h x y", h=8, x=32, y=32)
        wdst = qaug[32:64, :].rearrange("p (h x y) -> p h x y", h=8, x=32, y=32)
        hdst = qaug[64:96, :].rearrange("p (h x y) -> p h x y", h=8, x=32, y=32)

        # ================= Phase A =================
        with ExitStack() as actx:
            psA = actx.enter_context(tc.tile_pool(name="psA", bufs=2, space="PSUM"))

            # filler matmuls woven between transposes / rel batches keep the
            # HAM clock gate open (transposes and sparse small matmuls alone
            # leave the PE at 1.2 GHz). Fillers borrow idle pool slots.
            fillA = psA.tile([32, 1024], F32, tag="rel", bufs=2, name="fillA")

            def filler(ft, n=2):
                for _ in range(n):
                    nc.tensor.matmul(
                        out=ft[0:32, 0:128], lhsT=identb[0:32, 0:32],
                        rhs=identb[0:32, :], start=True, stop=True,
                    )

            # qT/kT psum tiles (kr transposes borrow two slots first)
            pts = {}
            for kind in range(2):
                for g in range(2):
                    pts[(kind, g)] = psA.tile([128, HW], BF16, tag="tps",
                                              bufs=4, name=f"pt{kind}{g}")

            # key_rel transposes: krw [63,32] -> krwT [32,63]. Borrow the tail
            # (chunk-7 region) of two pt slots — those transposes run last,
            # long after the krT copies drain.
            for srct, dst, pt in ((krw_sb, krwT, pts[(0, 0)]),
                                  (krh_sb, krhT, pts[(0, 1)])):
                nc.tensor.transpose(
                    out=pt[0:32, 896:959], in_=srct[:],
                    identity=identb[0:63, 0:63]
                )
                nc.vector.tensor_copy(out=dst[:], in_=pt[0:32, 896:959])

            # qT/kT: PE-transpose 4-head groups per input chunk as it lands,
            # fillers between, then DVE copies + partition-scatter DMAs.
            for c in range(8):
                for kind in range(2):
                    for g in range(2):
                        nc.tensor.transpose(
                            out=pts[(kind, g)][:, 128 * c:128 * c + 128],
                            in_=x_sb[:, 768 * c + 256 * kind + 128 * g:
                                     768 * c + 256 * kind + 128 * g + 128],
                            identity=identb[:, 0:128],
                        )
                filler(fillA, 2)
            for kind in range(2):
                dstt = qaug if kind == 0 else kaug
                for g in range(2):
                    pt = pts[(kind, g)]
                    scr = tscr[:, HW * (2 * kind + g):HW * (2 * kind + g) + HW]
                    if kind == 0:
                        nc.vector.tensor_scalar_mul(out=scr, in0=pt[:], scalar1=QSCALE)
                    else:
                        nc.vector.tensor_copy(out=scr, in_=pt[:])
                    # partition-scatter DMAs split across two issue queues
                    # (sync + gpsimd; NOT scalar - issues there would block
                    # the ACT rel-logit copies behind them in queue order)
                    for hh in range(4):
                        h = 4 * g + hh
                        eng = nc.sync if hh % 2 == 0 else nc.gpsimd
                        eng.dma_start(
                            out=dstt[0:32, HW * h:HW * h + HW],
                            in_=tscr[32 * hh:32 * hh + 32,
                                     HW * (2 * kind + g):HW * (2 * kind + g) + HW],
                        )
            emit_onehot_replication()

            # rel logits, batched: per 4 x-positions (H) / y-positions (W)
            # one [32, 1024] psum + one wide 4D-AP copy. Fillers (into a
            # retired pt slot) keep the PE warm through the copy waits.
            fillB = psA.tile([128, HW], BF16, tag="tps", bufs=4, name="fillB")
            fillBf = fillB[:].bitcast(F32)
            for t in range(8):
                pR = psA.tile([32, 1024], F32, tag="rel", bufs=2)
                pv = pR[:].rearrange("p (i h y) -> p i h y", i=4, h=8, y=32)
                for xi in range(4):
                    x = 4 * t + xi
                    nc.tensor.matmul(
                        out=pv[:, xi, :, :],
                        lhsT=krhT[:, 31 - x:63 - x],
                        rhs=qa4[:, :, x, :],
                        start=True, stop=True,
                    )
                # split each batch copy across both engines (heads 0-3 DVE,
                # 4-7 ACT) so the copy chain doesn't serialize the batches
                dst = hdst[:, :, 4 * t:4 * t + 4, :]
                src = pv[:].rearrange("p i h y -> p h i y")
                nc.vector.tensor_copy(out=dst[:, 0:4], in_=src[:, 0:4])
                nc.scalar.copy(out=dst[:, 4:8], in_=src[:, 4:8])
                filler(fillBf, 2)
            for t in range(8):
                pR = psA.tile([32, 1024], F32, tag="rel", bufs=2)
                pw = pR[:].rearrange("p (i h x) -> p i h x", i=4, h=8, x=32)
                for yi in range(4):
                    y = 4 * t + yi
                    nc.tensor.matmul(
                        out=pw[:, yi, :, :],
                        lhsT=krwT[:, 31 - y:63 - y],
                        rhs=qa4[:, :, :, y],
                        start=True, stop=True,
                    )
                dst = wdst[:, :, :, 4 * t:4 * t + 4]
                src = pw[:].rearrange("p i h x -> p h x i")
                nc.vector.tensor_copy(out=dst[:, 0:6], in_=src[:, 0:6])
                nc.scalar.copy(out=dst[:, 6:8], in_=src[:, 6:8])
                filler(fillBf, 2)

            # V chunks into v1 via SBUF->SBUF DMAs (64B runs)
            v1v = v1[:].rearrange("p (h c e) -> p c h e", h=8, c=8, e=33)
            for c in range(8):
                nc.sync.dma_start(
                    out=v1v[:, c, :, 0:32],
                    in_=x_sb[:, 768 * c + 512:768 * c + 768]
                        .rearrange("p (h e) -> p h e", h=8),
                )

        # ================= Phase B: attention =================
        with ExitStack() as bctx:
            psB = bctx.enter_context(tc.tile_pool(name="psB", bufs=1, space="PSUM"))
            sbW = bctx.enter_context(tc.tile_pool(name="sbW", bufs=3))
            sbA = bctx.enter_context(tc.tile_pool(name="sbA", bufs=2))
            sbR = bctx.enter_context(tc.tile_pool(name="sbR", bufs=3))

            def pair_chunks(hp, out_cb=None):
                """QK -> exp -> AV for the two heads of pair hp; returns the
                att psum -> SBUF staging tiles (copies emitted here so the
                att psum slots free up for the next pair ASAP). out_cb(i)
                (i=0..15) interleaves the previous pair's output tail one
                transpose per chunk so it never blocks the PE FIFO."""
                att = [psB.tile([97, 512], F32, tag=f"att{e}", bufs=1,
                                name=f"att{e}")
                       for e in range(2)]
                for hh in range(2):
                    h = 2 * hp + hh
                    pb = 64 * hh
                    for c in range(8):
                        if out_cb is not None:
                            out_cb(8 * hh + c)
                        s_ps = psB.tile([128, HW], F32, tag="s", bufs=2)
                        for e in range(2):
                            nc.tensor.matmul(
                                out=s_ps[:, 512 * e:512 * e + 512],
                                lhsT=kaug[:, HW * h + 128 * c:HW * h + 128 * c + 128],
                                rhs=qaug[:, HW * h + 512 * e:HW * h + 512 * e + 512],
                                start=True, stop=True,
                            )
                        wexp = sbW.tile([128, HW], BF16, tag="wexp")
                        if c in DVE_CHUNKS:
                            nc.vector.tensor_scalar(
                                out=wexp[:].bitcast(I16),
                                in0=s_ps[:],
                                scalar1=SCH_A, scalar2=SCH_B,
                                op0=MULT, op1=ADD,
                            )
                        else:
                            nc.scalar.activation(out=wexp[:], in_=s_ps[:], func=EXP)
                        for e in range(2):
                            nc.tensor.matmul(
                                out=att[e][pb:pb + 33, :],
                                lhsT=v1[:, 264 * h + 33 * c:264 * h + 33 * c + 33],
                                rhs=wexp[:, 512 * e:512 * e + 512],
                                start=(c == 0), stop=(c == 7),
                            )
                att_sb = []
                for e in range(2):
                    asb = sbA.tile([97, 512], F32, tag="attsb", bufs=4,
                                   name=f"attsb{e}")
                    nc.vector.tensor_copy(out=asb[:], in_=att[e][:])
                    att_sb.append(asb)
                return att_sb

            def pair_output_step(hp, att_sb, i):
                """One transpose + scale step (i=0..7) of pair hp's tail."""
                e, ci = i // 4, i % 4
                c = 4 * e + ci
                ot = psB.tile([128, 97], F32, tag="ot", bufs=2)
                nc.tensor.transpose(
                    out=ot[:],
                    in_=att_sb[e][:, 128 * ci:128 * ci + 128],
                    identity=identf[0:97, 0:97],
                )
                rc = sbR.tile([128, 2], F32, tag="rc")
                nc.vector.reciprocal(out=rc[:], in_=ot[:, 32:97:64])
                for hh in range(2):
                    nc.vector.tensor_scalar_mul(
                        out=out_sb[:, 256 * c + 64 * hp + 32 * hh:
                                   256 * c + 64 * hp + 32 * hh + 32],
                        in0=ot[:, 64 * hh:64 * hh + 32],
                        scalar1=rc[:, hh:hh + 1],
                    )
                if i == 7:
                    # per-pair output DMA (cols 64hp..+63 of each 256-block)
                    nc.sync.dma_start(
                        out=out_d[:].rearrange("(c p) d -> p c d", p=128)
                            [:, :, 64 * hp:64 * hp + 64],
                        in_=out_sb[:].rearrange("p (c d) -> p c d", c=8)
                            [:, :, 64 * hp:64 * hp + 64],
                    )

            pending = None
            for hp in range(NH // 2):
                prev = pending

                def out_cb(i, p=prev):
                    # two back-to-back transposes every 4th chunk (paired
                    # transposes share the pipeline drain)
                    if p is not None and i % 4 == 0:
                        pair_output_step(p[0], p[1], i // 2)
                        pair_output_step(p[0], p[1], i // 2 + 1)

                att_sb = pair_chunks(hp, out_cb=out_cb if prev else None)
                pending = (hp, att_sb)
            for i in range(8):
                pair_output_step(pending[0], pending[1], i)
    if not nc.is_finalized():
        nc.finalize()
    return nc


_NC = None


def _ensure_axon_hooks_module():
    """bass_utils imports antenv.axon_hooks unconditionally when trace=True;
    this image's antenv lacks it. Provide a stub so tracing degrades to
    no-trace instead of crashing (a real hook can be set by a profiler)."""
    import types

    if "antenv.axon_hooks" in sys.modules:
        return
    try:
        import antenv.axon_hooks  # noqa: F401
        return
    except ImportError:
        pass
    try:
        import antenv
    except ImportError:
        return
    m = types.ModuleType("antenv.axon_hooks")
    m._hook = None
    m.get_axon_ntff_profile_hook = lambda: m._hook
    m.set_axon_ntff_profile_hook = lambda h: setattr(m, "_hook", h)
    sys.modules["antenv.axon_hooks"] = m
    antenv.axon_hooks = m


def kernel(**inputs):
    global _NC
    x = np.ascontiguousarray(np.asarray(inputs["inputs"], dtype=np.float32))
    krw = np.ascontiguousarray(np.asarray(inputs["key_rel_w"], dtype=np.float32))
    krh = np.ascontiguousarray(np.asarray(inputs["key_rel_h"], dtype=np.float32))
    assert x.shape == (8, 32, 32, 768), x.shape
    assert int(inputs["dk"]) == 256 and int(inputs["dv"]) == 256
    assert int(inputs["Nh"]) == 8

    if _NC is None:
        _NC = build_nc()
    _ensure_axon_hooks_module()
    from concourse.bass_utils import run_bass_kernel_spmd

    in_maps = [
        {
            "xa": x[b].reshape(HW, CH)[:HW // 2],
            "xb": x[b].reshape(HW, CH)[HW // 2:],
            "krw": krw,
            "krh": krh,
        }
        for b in range(8)
    ]
    res = run_bass_kernel_spmd(_NC, in_maps, list(range(8)))
    kernel.last_result = res
    out = np.stack([res.results[b]["out"].reshape(32, 32, 256) for b in range(8)], 0)
    return out


if __name__ == "__main__":
    nc = build_nc()
    print("built ok")


# revision 37
# speedup vs baseline: 1.0857x; 1.0252x over previous
"""Trainium2 Bass kernel for nn_AttentionAugmentation (v2).

Attention with 2D relative-position logits. B=8, H=W=32, dk=dv=256, Nh=8.
Sharding: data-parallel over batch (one batch per NeuronCore, 8 cores).

Per-core algorithm (one batch, 8 heads of 1024x1024 attention, dkh=32):
  - inputs loaded as three column-split casting DMAs (q, k, v) so the q
    transposes / rel-logit matmuls start ~3us in, before k/v land.
  - PE warm-up with real matmuls (transposes don't open the HAM clock gate).
  - q/k PE-transposed in 4-head groups -> DVE copy (q scaled) -> partition-
    scatter DMAs into rows 0-31 of the augmented operands qaug/kaug.
  - relative logits folded into the main QK matmul via a 96-row augmented
    contraction: rows 32-63 (WRELT | one-hot of key y2), rows 64-95
    (HRELT | one-hot of key x2). WRELT/HRELT built with one small matmul
    per (y or x, 4-batch) over all heads; psum batched [32,1024] and copied
    with wide 4D-AP copies split across ACT/DVE (the old per-slice scalar
    copies serialized the whole front half of the kernel).
  - S^T = kaug^T @ qaug per 128-key chunk; exp split between ScalarE
    (true exp) and VectorE (Schraudolph: (int16)(A*S + B) bitcast as bf16
    approximates e^S to ~3%; C tuned for zero mean bias so ACT- and
    DVE-exp'd key chunks agree); AV with lhsT=[V | 1] giving attn^T rows
    plus softmax denominators.
  - attn^T PE-transposed back per 128-query chunk, scaled by reciprocal
    denominators into out_sb; per-pair output DMAs overlap the tail.
Matmul operands bf16 (fp32 PSUM accumulation).
"""
import sys

sys.path.insert(0, "/opt/trn_rl_repo")

from contextlib import ExitStack

import numpy as np

import concourse.bass as bass
from concourse import bacc
import concourse.mybir as mybir
from concourse import masks
from concourse.tile import TileContext

HW = 1024
CH = 768
NH = 8
F32 = mybir.dt.float32
BF16 = mybir.dt.bfloat16
I16 = mybir.dt.int16
EXP = mybir.ActivationFunctionType.Exp
MULT = mybir.AluOpType.mult
ADD = mybir.AluOpType.add
QSCALE = float((256 / 8) ** -0.5)
# Schraudolph exp for bf16: (int16)(A*x + B) bits ~= bf16(e^x).
SCH_A = 184.6649652337873      # 2^7 / ln(2)
SCH_B = 16256.0 - 7.0          # 127 * 2^7 + C (C=-7: zero mean ratio bias)
DVE_CHUNKS = (3, 7)            # key chunks exp'd on VectorE (rest ScalarE)


def build_nc():
    nc = bacc.Bacc()
    # input split in two halves: a single [1024, 768] parameter makes the
    # axon-pjrt reshard program's dynamic-slice exceed a 16-bit semaphore
    # field in neuronx-cc (25MB concat across 8 cores), crashing walrus.
    xa_d = nc.declare_dram_parameter("xa", [HW // 2, CH], F32, isOutput=False)
    xb_d = nc.declare_dram_parameter("xb", [HW // 2, CH], F32, isOutput=False)
    krw_d = nc.declare_dram_parameter("krw", [63, 32], F32, isOutput=False)
    krh_d = nc.declare_dram_parameter("krh", [63, 32], F32, isOutput=False)
    out_d = nc.declare_dram_parameter("out", [HW, 256], F32, isOutput=True)

    with ExitStack() as octx:
        tc = octx.enter_context(TileContext(nc))
        sb = octx.enter_context(tc.tile_pool(name="persist", bufs=1))

        x_sb = sb.tile([128, 6144], BF16)       # (c, e): chunk c, channel e
        qaug = sb.tile([96, NH * HW], BF16)     # per head h: cols 1024h + (32x + y)
        kaug = sb.tile([96, NH * HW], BF16)
        v1 = sb.tile([128, NH * 8 * 33], BF16)  # per (h,c): 33 cols = V chunk | ones
        tscr = sb.tile([128, 4 * HW], BF16)     # transpose scratch (kind, group)
        out_sb = sb.tile([128, 8 * 256], F32)   # col 256c + 32h + d
        identb = sb.tile([128, 128], BF16)
        identf = sb.tile([128, 128], F32)
        krw_sb = sb.tile([63, 32], BF16)
        krh_sb = sb.tile([63, 32], BF16)
        krwT = sb.tile([32, 63], BF16)
        krhT = sb.tile([32, 63], BF16)

        # ---- identity first (gates PE warm-up), then input DMAs ----
        masks.make_identity(nc, identb[:])

        # small rel tables first (krT transposes consume them immediately),
        # then full-bandwidth contiguous chunk loads. Split across two DMA
        # queues: even chunks SWDGE-cast on gpsimd, odd chunks fp32 on sync
        # (HWDGE can't cast) + DVE bf16 casts.
        nc.gpsimd.dma_start(out=krw_sb[:], in_=krw_d[:])
        nc.gpsimd.dma_start(out=krh_sb[:], in_=krh_d[:])
        x32_sb = sb.tile([128, 4 * 768], F32)
        for c in range(8):
            src_d = xa_d if c < 4 else xb_d
            cc = c % 4
            if c % 2 == 0:
                nc.gpsimd.dma_start(
                    out=x_sb[:, 768 * c:768 * c + 768],
                    in_=src_d[128 * cc:128 * cc + 128, :],
                )
            else:
                nc.sync.dma_start(
                    out=x32_sb[:, 768 * (c // 2):768 * (c // 2) + 768],
                    in_=src_d[128 * cc:128 * cc + 128, :],
                )
        for c in (1, 3, 5, 7):
            nc.vector.tensor_copy(
                out=x_sb[:, 768 * c:768 * c + 768],
                in_=x32_sb[:, 768 * (c // 2):768 * (c // 2) + 768],
            )

        # ---- remaining constants (gpsimd queue, after DMA issues) ----
        masks.make_identity(nc, identf[:])
        nc.gpsimd.memset(kaug[32:64, 0:HW], 0.0)
        nc.gpsimd.memset(kaug[64:96, 0:HW], 0.0)
        # one-hot blocks, head-0 only: rows 32-63: [y2(k)==j], rows 64-95:
        # [x2(k)==j]; key col = 32*x2 + y2. DMA-replicated to heads 1-7.
        nc.gpsimd.affine_select(
            out=kaug[32:64, 0:HW].rearrange("p (x y) -> p x y", x=32, y=32),
            in_=kaug[32:64, 0:HW].rearrange("p (x y) -> p x y", x=32, y=32),
            compare_op=mybir.AluOpType.not_equal,
            fill=1.0,
            base=0,
            pattern=[[0, 32], [-1, 32]],
            channel_multiplier=1,
        )
        nc.gpsimd.affine_select(
            out=kaug[64:96, 0:HW].rearrange("p (x y) -> p x y", x=32, y=32),
            in_=kaug[64:96, 0:HW].rearrange("p (x y) -> p x y", x=32, y=32),
            compare_op=mybir.AluOpType.not_equal,
            fill=1.0,
            base=0,
            pattern=[[-1, 32], [0, 32]],
            channel_multiplier=1,
        )
        nc.gpsimd.memset(v1[:], 1.0)

        def emit_onehot_replication():
            # one-hot replication on the gpsimd DMA queue, emitted AFTER the
            # q/k partition-scatters so its 14 issues don't delay them
            # (needed only by the first QK matmul, much later).
            for base in (32, 64):
                n = HW
                while n < NH * HW:
                    rep = min(n, NH * HW - n)
                    nc.gpsimd.dma_start(
                        out=kaug[base:base + 32, n:n + rep],
                        in_=kaug[base:base + 32, 0:rep],
                    )
                    n += rep

        qa4 = qaug[0:32, :].rearrange("p (h x y) -> p h x y", h=8, x=32, y=32)
        wdst = qaug[32:64, :].rearrange("p (h x y) -> p h x y", h=8, x=32, y=32)
        hdst = qaug[64:96, :].rearrange("p (h x y) -> p h x y", h=8, x=32, y=32)

        # ================= Phase A =================
        with ExitStack() as actx:
            psA = actx.enter_context(tc.tile_pool(name="psA", bufs=2, space="PSUM"))

            # filler matmuls woven between transposes / rel batches keep the
            # HAM clock gate open (transposes and sparse small matmuls alone
            # leave the PE at 1.2 GHz). Fillers borrow idle pool slots.
            fillA = psA.tile([32, 512], F32, tag="rel", bufs=4, name="fillA")

            def filler(ft, n=2):
                for _ in range(n):
                    nc.tensor.matmul(
                        out=ft[0:32, 0:128], lhsT=identb[0:32, 0:32],
                        rhs=identb[0:32, :], start=True, stop=True,
                    )

            # qT/kT psum tiles (kr transposes borrow two slots first)
            pts = {}
            for kind in range(2):
                for g in range(2):
                    pts[(kind, g)] = psA.tile([128, HW], BF16, tag="tps",
                                              bufs=4, name=f"pt{kind}{g}")

            # key_rel transposes: krw [63,32] -> krwT [32,63]. Borrow the tail
            # (chunk-7 region) of two pt slots — those transposes run last,
            # long after the krT copies drain.
            for srct, dst, pt in ((krw_sb, krwT, pts[(0, 0)]),
                                  (krh_sb, krhT, pts[(0, 1)])):
                nc.tensor.transpose(
                    out=pt[0:32, 896:959], in_=srct[:],
                    identity=identb[0:63, 0:63]
                )
                nc.vector.tensor_copy(out=dst[:], in_=pt[0:32, 896:959])

            # qT/kT: PE-transpose 4-head groups per input chunk as it lands,
            # fillers between, then DVE copies + partition-scatter DMAs.
            for c in range(8):
                for kind in range(2):
                    for g in range(2):
                        nc.tensor.transpose(
                            out=pts[(kind, g)][:, 128 * c:128 * c + 128],
                            in_=x_sb[:, 768 * c + 256 * kind + 128 * g:
                                     768 * c + 256 * kind + 128 * g + 128],
                            identity=identb[:, 0:128],
                        )
                filler(fillA, 2)
            for kind in range(2):
                dstt = qaug if kind == 0 else kaug
                for g in range(2):
                    pt = pts[(kind, g)]
                    scr = tscr[:, HW * (2 * kind + g):HW * (2 * kind + g) + HW]
                    if kind == 0:
                        nc.vector.tensor_scalar_mul(out=scr, in0=pt[:], scalar1=QSCALE)
                    else:
                        nc.vector.tensor_copy(out=scr, in_=pt[:])
                    # partition-scatter DMAs split across two issue queues
                    # (sync + gpsimd; NOT scalar - issues there would block
                    # the ACT rel-logit copies behind them in queue order)
                    for hh in range(4):
                        h = 4 * g + hh
                        eng = nc.sync if hh % 2 == 0 else nc.gpsimd
                        eng.dma_start(
                            out=dstt[0:32, HW * h:HW * h + HW],
                            in_=tscr[32 * hh:32 * hh + 32,
                                     HW * (2 * kind + g):HW * (2 * kind + g) + HW],
                        )
            emit_onehot_replication()

            # rel logits, batched: per 4 x-positions (H) / y-positions (W)
            # one [32, 1024] psum + one wide 4D-AP copy. Fillers (into a
            # retired pt slot) keep the PE warm through the copy waits.
            fillB = psA.tile([128, HW], BF16, tag="tps", bufs=4, name="fillB")
            fillBf = fillB[:].bitcast(F32)
            # 16 fine batches x 4 psum buffers per direction: copies pipeline
            # across both engines instead of serializing the batch chain
            for t in range(16):
                pR = psA.tile([32, 512], F32, tag="rel", bufs=4)
                pv = pR[:].rearrange("p (i h y) -> p i h y", i=2, h=8, y=32)
                for xi in range(2):
                    x = 2 * t + xi
                    nc.tensor.matmul(
                        out=pv[:, xi, :, :],
                        lhsT=krhT[:, 31 - x:63 - x],
                        rhs=qa4[:, :, x, :],
                        start=True, stop=True,
                    )
                dst = hdst[:, :, 2 * t:2 * t + 2, :]
                src = pv[:].rearrange("p i h y -> p h i y")
                if t % 2 == 0:
                    nc.vector.tensor_copy(out=dst, in_=src)
                else:
                    nc.scalar.copy(out=dst, in_=src)
                if t % 2 == 0:
                    filler(fillBf, 2)
            for t in range(16):
                pR = psA.tile([32, 512], F32, tag="rel", bufs=4)
                pw = pR[:].rearrange("p (i h x) -> p i h x", i=2, h=8, x=32)
                for yi in range(2):
                    y = 2 * t + yi
                    nc.tensor.matmul(
                        out=pw[:, yi, :, :],
                        lhsT=krwT[:, 31 - y:63 - y],
                        rhs=qa4[:, :, :, y],
                        start=True, stop=True,
                    )
                dst = wdst[:, :, :, 2 * t:2 * t + 2]
                src = pw[:].rearrange("p i h x -> p h x i")
                if t % 3 == 2:
                    nc.scalar.copy(out=dst, in_=src)
                else:
                    nc.vector.tensor_copy(out=dst, in_=src)
                if t % 2 == 0:
                    filler(fillBf, 2)

            # V chunks into v1 via SBUF->SBUF DMAs (64B runs)
            v1v = v1[:].rearrange("p (h c e) -> p c h e", h=8, c=8, e=33)
            for c in range(8):
                nc.sync.dma_start(
                    out=v1v[:, c, :, 0:32],
                    in_=x_sb[:, 768 * c + 512:768 * c + 768]
                        .rearrange("p (h e) -> p h e", h=8),
                )

        # ================= Phase B: attention =================
        with ExitStack() as bctx:
            psB = bctx.enter_context(tc.tile_pool(name="psB", bufs=1, space="PSUM"))
            sbW = bctx.enter_context(tc.tile_pool(name="sbW", bufs=3))
            sbA = bctx.enter_context(tc.tile_pool(name="sbA", bufs=2))
            sbR = bctx.enter_context(tc.tile_pool(name="sbR", bufs=3))

            def pair_chunks(hp, out_cb=None):
                """QK -> exp -> AV for the two heads of pair hp; returns the
                att psum -> SBUF staging tiles (copies emitted here so the
                att psum slots free up for the next pair ASAP). out_cb(i)
                (i=0..15) interleaves the previous pair's output tail one
                transpose per chunk so it never blocks the PE FIFO."""
                att = [psB.tile([97, 512], F32, tag=f"att{e}", bufs=1,
                                name=f"att{e}")
                       for e in range(2)]
                for hh in range(2):
                    h = 2 * hp + hh
                    pb = 64 * hh
                    for c in range(8):
                        if out_cb is not None:
                            out_cb(8 * hh + c)
                        s_ps = psB.tile([128, HW], F32, tag="s", bufs=2)
                        for e in range(2):
                            nc.tensor.matmul(
                                out=s_ps[:, 512 * e:512 * e + 512],
                                lhsT=kaug[:, HW * h + 128 * c:HW * h + 128 * c + 128],
                                rhs=qaug[:, HW * h + 512 * e:HW * h + 512 * e + 512],
                                start=True, stop=True,
                            )
                        wexp = sbW.tile([128, HW], BF16, tag="wexp")
                        if c in DVE_CHUNKS:
                            nc.vector.tensor_scalar(
                                out=wexp[:].bitcast(I16),
                                in0=s_ps[:],
                                scalar1=SCH_A, scalar2=SCH_B,
                                op0=MULT, op1=ADD,
                            )
                        else:
                            nc.scalar.activation(out=wexp[:], in_=s_ps[:], func=EXP)
                        for e in range(2):
                            nc.tensor.matmul(
                                out=att[e][pb:pb + 33, :],
                                lhsT=v1[:, 264 * h + 33 * c:264 * h + 33 * c + 33],
                                rhs=wexp[:, 512 * e:512 * e + 512],
                                start=(c == 0), stop=(c == 7),
                            )
                att_sb = []
                for e in range(2):
                    asb = sbA.tile([97, 512], F32, tag="attsb", bufs=4,
                                   name=f"attsb{e}")
                    nc.vector.tensor_copy(out=asb[:], in_=att[e][:])
                    att_sb.append(asb)
                return att_sb

            def pair_output_step(hp, att_sb, i):
                """One transpose + scale step (i=0..7) of pair hp's tail."""
                e, ci = i // 4, i % 4
                c = 4 * e + ci
                ot = psB.tile([128, 97], F32, tag="ot", bufs=2)
                nc.tensor.transpose(
                    out=ot[:],
                    in_=att_sb[e][:, 128 * ci:128 * ci + 128],
                    identity=identf[0:97, 0:97],
                )
                rc = sbR.tile([128, 2], F32, tag="rc")
                nc.vector.reciprocal(out=rc[:], in_=ot[:, 32:97:64])
                for hh in range(2):
                    nc.vector.tensor_scalar_mul(
                        out=out_sb[:, 256 * c + 64 * hp + 32 * hh:
                                   256 * c + 64 * hp + 32 * hh + 32],
                        in0=ot[:, 64 * hh:64 * hh + 32],
                        scalar1=rc[:, hh:hh + 1],
                    )
                if i == 7:
                    # per-pair output DMA (cols 64hp..+63 of each 256-block)
                    nc.sync.dma_start(
                        out=out_d[:].rearrange("(c p) d -> p c d", p=128)
                            [:, :, 64 * hp:64 * hp + 64],
                        in_=out_sb[:].rearrange("p (c d) -> p c d", c=8)
                            [:, :, 64 * hp:64 * hp + 64],
                    )

            pending = None
            for hp in range(NH // 2):
                prev = pending

                def out_cb(i, p=prev):
                    # two back-to-back transposes every 4th chunk (paired
                    # transposes share the pipeline drain)
                    if p is not None and i % 4 == 0:
                        pair_output_step(p[0], p[1], i // 2)
                        pair_output_step(p[0], p[1], i // 2 + 1)

                att_sb = pair_chunks(hp, out_cb=out_cb if prev else None)
                pending = (hp, att_sb)
            for i in range(8):
                pair_output_step(pending[0], pending[1], i)
    if not nc.is_finalized():
        nc.finalize()
    return nc


_NC = None


def _ensure_axon_hooks_module():
    """bass_utils imports antenv.axon_hooks unconditionally when trace=True;
    this image's antenv lacks it. Provide a stub so tracing degrades to
    no-trace instead of crashing (a real hook can be set by a profiler)."""
    import types

    if "antenv.axon_hooks" in sys.modules:
        return
    try:
        import antenv.axon_hooks  # noqa: F401
        return
    except ImportError:
        pass
    try:
        import antenv
    except ImportError:
        return
    m = types.ModuleType("antenv.axon_hooks")
    m._hook = None
    m.get_axon_ntff_profile_hook = lambda: m._hook
    m.set_axon_ntff_profile_hook = lambda h: setattr(m, "_hook", h)
    sys.modules["antenv.axon_hooks"] = m
    antenv.axon_hooks = m


def kernel(**inputs):
    global _NC
    x = np.ascontiguousarray(np.asarray(inputs["inputs"], dtype=np.float32))
    krw = np.ascontiguousarray(np.asarray(inputs["key_rel_w"], dtype=np.float32))
    krh = np.ascontiguousarray(np.asarray(inputs["key_rel_h"], dtype=np.float32))
    assert x.shape == (8, 32, 32, 768), x.shape
    assert int(inputs["dk"]) == 256 and int(inputs["dv"]) == 256
    assert int(inputs["Nh"]) == 8

    if _NC is None:
        _NC = build_nc()
    _ensure_axon_hooks_module()
    from concourse.bass_utils import run_bass_kernel_spmd

    in_maps = [
        {
            "xa": x[b].reshape(HW, CH)[:HW // 2],
            "xb": x[b].reshape(HW, CH)[HW // 2:],
            "krw": krw,
            "krh": krh,
        }
        for b in range(8)
    ]
    res = run_bass_kernel_spmd(_NC, in_maps, list(range(8)))
    kernel.last_result = res
    out = np.stack([res.results[b]["out"].reshape(32, 32, 256) for b in range(8)], 0)
    return out


if __name__ == "__main__":
    nc = build_nc()
    print("built ok")


# revision 43
# speedup vs baseline: 1.1259x; 1.0370x over previous
"""Trainium2 Bass kernel for nn_AttentionAugmentation (v2).

Attention with 2D relative-position logits. B=8, H=W=32, dk=dv=256, Nh=8.
Sharding: data-parallel over batch (one batch per NeuronCore, 8 cores).

Per-core algorithm (one batch, 8 heads of 1024x1024 attention, dkh=32):
  - inputs loaded as three column-split casting DMAs (q, k, v) so the q
    transposes / rel-logit matmuls start ~3us in, before k/v land.
  - PE warm-up with real matmuls (transposes don't open the HAM clock gate).
  - q/k PE-transposed in 4-head groups -> DVE copy (q scaled) -> partition-
    scatter DMAs into rows 0-31 of the augmented operands qaug/kaug.
  - relative logits folded into the main QK matmul via a 96-row augmented
    contraction: rows 32-63 (WRELT | one-hot of key y2), rows 64-95
    (HRELT | one-hot of key x2). WRELT/HRELT built with one small matmul
    per (y or x, 4-batch) over all heads; psum batched [32,1024] and copied
    with wide 4D-AP copies split across ACT/DVE (the old per-slice scalar
    copies serialized the whole front half of the kernel).
  - S^T = kaug^T @ qaug per 128-key chunk; exp split between ScalarE
    (true exp) and VectorE (Schraudolph: (int16)(A*S + B) bitcast as bf16
    approximates e^S to ~3%; C tuned for zero mean bias so ACT- and
    DVE-exp'd key chunks agree); AV with lhsT=[V | 1] giving attn^T rows
    plus softmax denominators.
  - attn^T PE-transposed back per 128-query chunk, scaled by reciprocal
    denominators into out_sb; per-pair output DMAs overlap the tail.
Matmul operands bf16 (fp32 PSUM accumulation).
"""
import sys

sys.path.insert(0, "/opt/trn_rl_repo")

from contextlib import ExitStack

import numpy as np

import concourse.bass as bass
from concourse import bacc
import concourse.mybir as mybir
from concourse import masks
from concourse.tile import TileContext

HW = 1024
CH = 768
NH = 8
F32 = mybir.dt.float32
BF16 = mybir.dt.bfloat16
I16 = mybir.dt.int16
EXP = mybir.ActivationFunctionType.Exp
MULT = mybir.AluOpType.mult
ADD = mybir.AluOpType.add
QSCALE = float((256 / 8) ** -0.5)
# Schraudolph exp for bf16: (int16)(A*x + B) bits ~= bf16(e^x).
SCH_A = 184.6649652337873      # 2^7 / ln(2)
SCH_B = 16256.0 - 7.0          # 127 * 2^7 + C (C=-7: zero mean ratio bias)
DVE_CHUNKS = (3, 7)            # key chunks exp'd on VectorE (rest ScalarE)


def build_nc():
    nc = bacc.Bacc()
    # input split in two halves: a single [1024, 768] parameter makes the
    # axon-pjrt reshard program's dynamic-slice exceed a 16-bit semaphore
    # field in neuronx-cc (25MB concat across 8 cores), crashing walrus.
    xa_d = nc.declare_dram_parameter("xa", [HW // 2, CH], F32, isOutput=False)
    xb_d = nc.declare_dram_parameter("xb", [HW // 2, CH], F32, isOutput=False)
    krw_d = nc.declare_dram_parameter("krw", [63, 32], F32, isOutput=False)
    krh_d = nc.declare_dram_parameter("krh", [63, 32], F32, isOutput=False)
    out_d = nc.declare_dram_parameter("out", [HW, 256], F32, isOutput=True)

    with ExitStack() as octx:
        tc = octx.enter_context(TileContext(nc))
        sb = octx.enter_context(tc.tile_pool(name="persist", bufs=1))

        x_sb = sb.tile([128, 6144], BF16)       # (c, e): chunk c, channel e
        qaug = sb.tile([96, NH * HW], BF16)     # per head h: cols 1024h + (32x + y)
        qaugT = sb.tile([32, NH * HW], BF16)    # qT rows, y-major: 1024h + (32y + x)
        kaug = sb.tile([96, NH * HW], BF16)
        v1 = sb.tile([128, NH * 8 * 33], BF16)  # per (h,c): 33 cols = V chunk | ones
        tscr = sb.tile([128, 4 * HW], BF16)     # transpose scratch (kind, group)
        tscr2 = sb.tile([128, 2 * HW], BF16)    # y-major q scratch (group)
        out_sb = sb.tile([128, 8 * 256], F32)   # col 256c + 32h + d
        identb = sb.tile([128, 128], BF16)
        identf = sb.tile([128, 128], F32)
        krw_sb = sb.tile([63, 32], BF16)
        krh_sb = sb.tile([63, 32], BF16)
        krwT = sb.tile([32, 63], BF16)
        krhT = sb.tile([32, 63], BF16)

        # ---- identity first (gates PE warm-up), then input DMAs ----
        masks.make_identity(nc, identb[:])

        # small rel tables first (krT transposes consume them immediately),
        # then full-bandwidth contiguous chunk loads. Split across two DMA
        # queues: even chunks SWDGE-cast on gpsimd, odd chunks fp32 on sync
        # (HWDGE can't cast) + DVE bf16 casts.
        nc.gpsimd.dma_start(out=krw_sb[:], in_=krw_d[:])
        nc.gpsimd.dma_start(out=krh_sb[:], in_=krh_d[:])
        x32_sb = sb.tile([128, 4 * 768], F32)
        for c in range(8):
            src_d = xa_d if c < 4 else xb_d
            cc = c % 4
            if c % 2 == 0:
                nc.gpsimd.dma_start(
                    out=x_sb[:, 768 * c:768 * c + 768],
                    in_=src_d[128 * cc:128 * cc + 128, :],
                )
            else:
                nc.sync.dma_start(
                    out=x32_sb[:, 768 * (c // 2):768 * (c // 2) + 768],
                    in_=src_d[128 * cc:128 * cc + 128, :],
                )
        for c in (1, 3, 5, 7):
            nc.vector.tensor_copy(
                out=x_sb[:, 768 * c:768 * c + 768],
                in_=x32_sb[:, 768 * (c // 2):768 * (c // 2) + 768],
            )

        # ---- remaining constants (gpsimd queue, after DMA issues) ----
        masks.make_identity(nc, identf[:])
        nc.gpsimd.memset(kaug[32:64, 0:HW], 0.0)
        nc.gpsimd.memset(kaug[64:96, 0:HW], 0.0)
        # one-hot blocks, head-0 only: rows 32-63: [y2(k)==j], rows 64-95:
        # [x2(k)==j]; key col = 32*x2 + y2. DMA-replicated to heads 1-7.
        nc.gpsimd.affine_select(
            out=kaug[32:64, 0:HW].rearrange("p (x y) -> p x y", x=32, y=32),
            in_=kaug[32:64, 0:HW].rearrange("p (x y) -> p x y", x=32, y=32),
            compare_op=mybir.AluOpType.not_equal,
            fill=1.0,
            base=0,
            pattern=[[0, 32], [-1, 32]],
            channel_multiplier=1,
        )
        nc.gpsimd.affine_select(
            out=kaug[64:96, 0:HW].rearrange("p (x y) -> p x y", x=32, y=32),
            in_=kaug[64:96, 0:HW].rearrange("p (x y) -> p x y", x=32, y=32),
            compare_op=mybir.AluOpType.not_equal,
            fill=1.0,
            base=0,
            pattern=[[-1, 32], [0, 32]],
            channel_multiplier=1,
        )
        nc.gpsimd.memset(v1[:], 1.0)

        def emit_onehot_replication():
            # one-hot replication on the gpsimd DMA queue, emitted AFTER the
            # q/k partition-scatters so its 14 issues don't delay them
            # (needed only by the first QK matmul, much later).
            for base in (32, 64):
                n = HW
                while n < NH * HW:
                    rep = min(n, NH * HW - n)
                    nc.gpsimd.dma_start(
                        out=kaug[base:base + 32, n:n + rep],
                        in_=kaug[base:base + 32, 0:rep],
                    )
                    n += rep

        qa4 = qaug[0:32, :].rearrange("p (h x y) -> p h x y", h=8, x=32, y=32)
        wdst = qaug[32:64, :].rearrange("p (h x y) -> p h x y", h=8, x=32, y=32)
        hdst = qaug[64:96, :].rearrange("p (h x y) -> p h x y", h=8, x=32, y=32)

        # ================= Phase A =================
        with ExitStack() as actx:
            psA = actx.enter_context(tc.tile_pool(name="psA", bufs=2, space="PSUM"))

            # filler matmuls woven between transposes / rel batches keep the
            # HAM clock gate open (transposes and sparse small matmuls alone
            # leave the PE at 1.2 GHz). Fillers borrow idle pool slots.
            fillA = psA.tile([32, 512], F32, tag="rel", bufs=4, name="fillA")

            def filler(ft, n=2):
                for _ in range(n):
                    nc.tensor.matmul(
                        out=ft[0:32, 0:128], lhsT=identb[0:32, 0:32],
                        rhs=identb[0:32, :], start=True, stop=True,
                    )

            # qT/kT psum tiles (kr transposes borrow two slots first)
            pts = {}
            for kind in range(2):
                for g in range(2):
                    pts[(kind, g)] = psA.tile([128, HW], BF16, tag="tps",
                                              bufs=4, name=f"pt{kind}{g}")

            # key_rel transposes: krw [63,32] -> krwT [32,63]. Borrow the tail
            # (chunk-7 region) of two pt slots — those transposes run last,
            # long after the krT copies drain.
            for srct, dst, pt in ((krw_sb, krwT, pts[(0, 0)]),
                                  (krh_sb, krhT, pts[(0, 1)])):
                nc.tensor.transpose(
                    out=pt[0:32, 896:959], in_=srct[:],
                    identity=identb[0:63, 0:63]
                )
                nc.vector.tensor_copy(out=dst[:], in_=pt[0:32, 896:959])

            # qT/kT: PE-transpose 4-head groups per input chunk as it lands,
            # fillers between, then DVE copies + partition-scatter DMAs.
            for c in range(8):
                for kind in range(2):
                    for g in range(2):
                        nc.tensor.transpose(
                            out=pts[(kind, g)][:, 128 * c:128 * c + 128],
                            in_=x_sb[:, 768 * c + 256 * kind + 128 * g:
                                     768 * c + 256 * kind + 128 * g + 128],
                            identity=identb[:, 0:128],
                        )
                filler(fillA, 2)
            for kind in range(2):
                dstt = qaug if kind == 0 else kaug
                for g in range(2):
                    pt = pts[(kind, g)]
                    scr = tscr[:, HW * (2 * kind + g):HW * (2 * kind + g) + HW]
                    if kind == 0:
                        nc.vector.tensor_scalar_mul(out=scr, in0=pt[:], scalar1=QSCALE)
                        # y-major copy of scaled qT: feeds the rel-W matmuls
                        # with a contiguous rhs (strided rhs streams ~3x slow)
                        nc.vector.tensor_scalar_mul(
                            out=tscr2[:, HW * g:HW * g + HW]
                                .rearrange("p (y x) -> p y x", y=32),
                            in0=pt[:].rearrange("p (x y) -> p y x", x=32),
                            scalar1=QSCALE,
                        )
                    else:
                        nc.vector.tensor_copy(out=scr, in_=pt[:])
                    # partition-scatter DMAs split across two issue queues
                    # (sync + gpsimd; NOT scalar - issues there would block
                    # the ACT rel-logit copies behind them in queue order)
                    for hh in range(4):
                        h = 4 * g + hh
                        eng = nc.sync if hh % 2 == 0 else nc.gpsimd
                        eng.dma_start(
                            out=dstt[0:32, HW * h:HW * h + HW],
                            in_=tscr[32 * hh:32 * hh + 32,
                                     HW * (2 * kind + g):HW * (2 * kind + g) + HW],
                        )
                        if kind == 0:
                            eng.dma_start(
                                out=qaugT[0:32, HW * h:HW * h + HW],
                                in_=tscr2[32 * hh:32 * hh + 32,
                                          HW * g:HW * g + HW],
                            )
            emit_onehot_replication()

            # rel logits, batched: per 4 x-positions (H) / y-positions (W)
            # one [32, 1024] psum + one wide 4D-AP copy. Fillers (into a
            # retired pt slot) keep the PE warm through the copy waits.
            fillB = psA.tile([128, HW], BF16, tag="tps", bufs=4, name="fillB")
            fillBf = fillB[:].bitcast(F32)
            # 16 fine batches x 4 psum buffers per direction: copies pipeline
            # across both engines instead of serializing the batch chain
            for t in range(16):
                pR = psA.tile([32, 512], F32, tag="rel", bufs=4)
                pv = pR[:].rearrange("p (i h y) -> p i h y", i=2, h=8, y=32)
                for xi in range(2):
                    x = 2 * t + xi
                    nc.tensor.matmul(
                        out=pv[:, xi, :, :],
                        lhsT=krhT[:, 31 - x:63 - x],
                        rhs=qa4[:, :, x, :],
                        start=True, stop=True,
                    )
                dst = hdst[:, :, 2 * t:2 * t + 2, :]
                src = pv[:].rearrange("p i h y -> p h i y")
                if t % 2 == 0:
                    nc.vector.tensor_copy(out=dst, in_=src)
                else:
                    nc.scalar.copy(out=dst, in_=src)
                if t % 2 == 0:
                    filler(fillBf, 2)
            qa4t = qaugT[0:32, :].rearrange("p (h y x) -> p h y x",
                                            h=8, y=32, x=32)
            for t in range(16):
                pR = psA.tile([32, 512], F32, tag="rel", bufs=4)
                pw = pR[:].rearrange("p (i h x) -> p i h x", i=2, h=8, x=32)
                for yi in range(2):
                    y = 2 * t + yi
                    nc.tensor.matmul(
                        out=pw[:, yi, :, :],
                        lhsT=krwT[:, 31 - y:63 - y],
                        rhs=qa4t[:, :, y, :],
                        start=True, stop=True,
                    )
                dst = wdst[:, :, :, 2 * t:2 * t + 2]
                src = pw[:].rearrange("p i h x -> p h x i")
                if t % 3 == 2:
                    nc.scalar.copy(out=dst, in_=src)
                else:
                    nc.vector.tensor_copy(out=dst, in_=src)
                if t % 2 == 0:
                    filler(fillBf, 2)

            # V chunks into v1 via SBUF->SBUF DMAs (64B runs)
            v1v = v1[:].rearrange("p (h c e) -> p c h e", h=8, c=8, e=33)
            for c in range(8):
                nc.sync.dma_start(
                    out=v1v[:, c, :, 0:32],
                    in_=x_sb[:, 768 * c + 512:768 * c + 768]
                        .rearrange("p (h e) -> p h e", h=8),
                )

        # ================= Phase B: attention =================
        with ExitStack() as bctx:
            psB = bctx.enter_context(tc.tile_pool(name="psB", bufs=1, space="PSUM"))
            sbW = bctx.enter_context(tc.tile_pool(name="sbW", bufs=3))
            sbA = bctx.enter_context(tc.tile_pool(name="sbA", bufs=2))
            sbR = bctx.enter_context(tc.tile_pool(name="sbR", bufs=3))

            def pair_chunks(hp, out_cb=None):
                """QK -> exp -> AV for the two heads of pair hp; returns the
                att psum -> SBUF staging tiles (copies emitted here so the
                att psum slots free up for the next pair ASAP). out_cb(i)
                (i=0..15) interleaves the previous pair's output tail one
                transpose per chunk so it never blocks the PE FIFO."""
                att = [psB.tile([97, 512], F32, tag=f"att{e}", bufs=1,
                                name=f"att{e}")
                       for e in range(2)]
                for hh in range(2):
                    h = 2 * hp + hh
                    pb = 64 * hh
                    for c in range(8):
                        if out_cb is not None:
                            out_cb(8 * hh + c)
                        s_ps = psB.tile([128, HW], F32, tag="s", bufs=2)
                        for e in range(2):
                            nc.tensor.matmul(
                                out=s_ps[:, 512 * e:512 * e + 512],
                                lhsT=kaug[:, HW * h + 128 * c:HW * h + 128 * c + 128],
                                rhs=qaug[:, HW * h + 512 * e:HW * h + 512 * e + 512],
                                start=True, stop=True,
                            )
                        wexp = sbW.tile([128, HW], BF16, tag="wexp")
                        if c in DVE_CHUNKS:
                            nc.vector.tensor_scalar(
                                out=wexp[:].bitcast(I16),
                                in0=s_ps[:],
                                scalar1=SCH_A, scalar2=SCH_B,
                                op0=MULT, op1=ADD,
                            )
                        else:
                            nc.scalar.activation(out=wexp[:], in_=s_ps[:], func=EXP)
                        for e in range(2):
                            nc.tensor.matmul(
                                out=att[e][pb:pb + 33, :],
                                lhsT=v1[:, 264 * h + 33 * c:264 * h + 33 * c + 33],
                                rhs=wexp[:, 512 * e:512 * e + 512],
                                start=(c == 0), stop=(c == 7),
                            )
                att_sb = []
                for e in range(2):
                    asb = sbA.tile([97, 512], F32, tag="attsb", bufs=4,
                                   name=f"attsb{e}")
                    nc.vector.tensor_copy(out=asb[:], in_=att[e][:])
                    att_sb.append(asb)
                return att_sb

            def pair_output_step(hp, att_sb, i):
                """One transpose + scale step (i=0..7) of pair hp's tail."""
                e, ci = i // 4, i % 4
                c = 4 * e + ci
                ot = psB.tile([128, 97], F32, tag="ot", bufs=2)
                nc.tensor.transpose(
                    out=ot[:],
                    in_=att_sb[e][:, 128 * ci:128 * ci + 128],
                    identity=identf[0:97, 0:97],
                )
                rc = sbR.tile([128, 2], F32, tag="rc")
                nc.vector.reciprocal(out=rc[:], in_=ot[:, 32:97:64])
                for hh in range(2):
                    nc.vector.tensor_scalar_mul(
                        out=out_sb[:, 256 * c + 64 * hp + 32 * hh:
                                   256 * c + 64 * hp + 32 * hh + 32],
                        in0=ot[:, 64 * hh:64 * hh + 32],
                        scalar1=rc[:, hh:hh + 1],
                    )
                if i == 7:
                    # per-pair output DMA (cols 64hp..+63 of each 256-block)
                    nc.sync.dma_start(
                        out=out_d[:].rearrange("(c p) d -> p c d", p=128)
                            [:, :, 64 * hp:64 * hp + 64],
                        in_=out_sb[:].rearrange("p (c d) -> p c d", c=8)
                            [:, :, 64 * hp:64 * hp + 64],
                    )

            pending = None
            for hp in range(NH // 2):
                prev = pending

                def out_cb(i, p=prev):
                    # two back-to-back transposes every 4th chunk (paired
                    # transposes share the pipeline drain)
                    if p is not None and i % 4 == 0:
                        pair_output_step(p[0], p[1], i // 2)
                        pair_output_step(p[0], p[1], i // 2 + 1)

                att_sb = pair_chunks(hp, out_cb=out_cb if prev else None)
                pending = (hp, att_sb)
            for i in range(8):
                pair_output_step(pending[0], pending[1], i)
    if not nc.is_finalized():
        nc.finalize()
    return nc


_NC = None


def _ensure_axon_hooks_module():
    """bass_utils imports antenv.axon_hooks unconditionally when trace=True;
    this image's antenv lacks it. Provide a stub so tracing degrades to
    no-trace instead of crashing (a real hook can be set by a profiler)."""
    import types

    if "antenv.axon_hooks" in sys.modules:
        return
    try:
        import antenv.axon_hooks  # noqa: F401
        return
    except ImportError:
        pass
    try:
        import antenv
    except ImportError:
        return
    m = types.ModuleType("antenv.axon_hooks")
    m._hook = None
    m.get_axon_ntff_profile_hook = lambda: m._hook
    m.set_axon_ntff_profile_hook = lambda h: setattr(m, "_hook", h)
    sys.modules["antenv.axon_hooks"] = m
    antenv.axon_hooks = m


def kernel(**inputs):
    global _NC
    x = np.ascontiguousarray(np.asarray(inputs["inputs"], dtype=np.float32))
    krw = np.ascontiguousarray(np.asarray(inputs["key_rel_w"], dtype=np.float32))
    krh = np.ascontiguousarray(np.asarray(inputs["key_rel_h"], dtype=np.float32))
    assert x.shape == (8, 32, 32, 768), x.shape
    assert int(inputs["dk"]) == 256 and int(inputs["dv"]) == 256
    assert int(inputs["Nh"]) == 8

    if _NC is None:
        _NC = build_nc()
    _ensure_axon_hooks_module()
    from concourse.bass_utils import run_bass_kernel_spmd

    in_maps = [
        {
            "xa": x[b].reshape(HW, CH)[:HW // 2],
            "xb": x[b].reshape(HW, CH)[HW // 2:],
            "krw": krw,
            "krh": krh,
        }
        for b in range(8)
    ]
    res = run_bass_kernel_spmd(_NC, in_maps, list(range(8)))
    kernel.last_result = res
    out = np.stack([res.results[b]["out"].reshape(32, 32, 256) for b in range(8)], 0)
    return out


if __name__ == "__main__":
    nc = build_nc()
    print("built ok")
